# revision 1
# baseline (speedup 1.0000x reference)
"""BinaryMoSLinear Trainium2 kernel (8-core SPMD, data-parallel over tokens).

Math (per reference):
    xf      = x.reshape(N, H)
    routing = softmax(xf @ gate_w.T)            # [N, E], E = 8
    in_s    = routing @ in_channel_scale        # [N, H]
    out_s   = routing @ out_channel_scale       # [N, O]
    out     = (xf * in_s) @ sign(weight).T * out_s + bias

Device factorization (division-free, all matmuls contract on partitions):
    expT[e, t]   = exp(logitsT[e, t])          (raw, unstabilized; bf16)
    den[t]       = sum_e expT[e, t]            (PE mm with a ones column)
    is_raw[h, t] = sum_e ics[e, h] expT[e, t]  (PE mm, natural ics layout)
    aT[h, t]     = bf16(xT[h, t] * is_raw)     (softmax denom factored out)
    main[t, o]   = sum_h aT[h, t] sign(w)[o, h]
    os_sb[t, o]  = (sum_e expT[e, t] ocs[e, o]) / den[t]^2
    out[t, o]    = main * os_sb + bias[o]

Each core gets 1024 tokens and the full weight; no collectives.  x, weight
and gate_w are transposed/permuted ON THE HOST during sharding (pure layout
transforms) into partition-major blocks, so every DMA moves >=16KB-contiguous
per-partition runs and the device program contains no PE transposes at all.
The PE queue is pure matmuls; sign(weight) runs on ACT from fp32 stripes.
"""

import numpy as np

import concourse.bass as bass
import concourse.mybir as mybir
from concourse import tile
from concourse.bass_utils import run_bass_kernel_spmd

F32 = mybir.dt.float32
BF16 = mybir.dt.bfloat16
AF = mybir.ActivationFunctionType
ALU = mybir.AluOpType

P = 128
E = 8
N_CORES = 8

# full problem: x [4, 2048, 4096], weight [4096, 4096]
FULL_B, FULL_S, FULL_H, FULL_O = 4, 2048, 4096, 4096
TOK = FULL_B * FULL_S // N_CORES  # 1024 tokens per core


# --------------------------------------------------------------------------
# This container's walrus build accepts at most ONE sync-wait command per
# instruction (DMA descriptors especially).  Tile's scheduler freely stacks
# several waits on one instruction, so rewrite the BIR JSON before compile:
# excess waits become single-wait NoOps immediately preceding the instruction
# on the same engine (program order => identical semantics).
_MAXW = 1


def _split_excess_waits(bir_json: bytes, maxw: int = _MAXW) -> bytes:
    import json as _json

    j = _json.loads(bir_json)
    ctr = 0
    for fn in j["functions"]:
        for blk in fn["blocks"]:
            new = []
            for inst in blk["instructions"]:
                si = inst.get("sync_info")
                if si:
                    waits = si.get("on_wait") or []
                    if len(waits) > maxw:
                        extra, keep = waits[:-maxw], waits[-maxw:]
                        for i in range(0, len(extra), maxw):
                            ctr += 1
                            nop = {
                                "name": f"I-wsplit-{ctr}",
                                "opcode": "NoOp",
                                "engine": inst["engine"],
                                "ins": [],
                                "outs": [],
                                "sync_info": {
                                    "on_wait": extra[i : i + maxw],
                                    "on_update": [],
                                },
                            }
                            if "debug" in inst:
                                nop["debug"] = inst["debug"]
                            new.append(nop)
                        si["on_wait"] = keep
                new.append(inst)
            blk["instructions"] = new
    return _json.dumps(j).encode()


def _install_wait_split():
    from concourse import bass2jax, bass_utils

    orig = bass_utils.compile_bir_kernel
    if getattr(orig, "_wait_split_wrapped", False):
        return

    def wrapped(bir_json, tmpdir, neff_name="file.neff"):
        return orig(_split_excess_waits(bir_json), tmpdir, neff_name)

    wrapped._wait_split_wrapped = True
    bass_utils.compile_bir_kernel = wrapped
    bass2jax.compile_bir_kernel = wrapped


_install_wait_split()
# --------------------------------------------------------------------------


def build_nc(tok=TOK, h=FULL_H, o=FULL_O):
    """Build the per-core Bass program (operands pre-permuted in DRAM)."""
    HC = h // P          # 128-wide h-chunks
    TB = tok // P        # 128-token blocks
    TH = tok // 512      # 512-token halves (gating granularity)
    ON = 512             # main-mm moving width / output stripe width
    OC = o // ON         # output-column chunks
    TG = 4               # concurrent main psum accumulations
    XP = 4               # x DMA parts per half (pipelines gating vs load)
    WP = 4               # w DMA/sign parts per output stripe
    assert tok % 512 == 0 and TB % TG == 0 and HC % XP == 0

    nc = bass.Bass("TRN2", target_bir_lowering=False, debug=False,
                   num_devices=N_CORES)

    # x halves, permuted to [p, hc, t'] (partition-major, contiguous)
    x0_d = nc.declare_dram_parameter("x0", [P, HC * 512], F32, isOutput=False)
    x1_d = nc.declare_dram_parameter("x1", [P, HC * 512], F32, isOutput=False)
    # w permuted to [p, oc, hc, o'] (partition-major, contiguous per stripe)
    wr_d = nc.declare_dram_parameter("wr", [P, OC * HC * ON], F32,
                                     isOutput=False)
    b_d = nc.declare_dram_parameter("bias", [o], F32, isOutput=False)
    # gwT host-permuted to [p, hc, e] (1KB-contiguous per partition; the
    # natural [h, E] layout DMAs as 4096 32-byte descriptors that clog
    # every engine right when x half 0 needs them)
    gwT_d = nc.declare_dram_parameter("gwr", [P, HC * E], F32, isOutput=False)
    ics_d = nc.declare_dram_parameter("ics", [E, h], F32, isOutput=False)
    ocs_d = nc.declare_dram_parameter("ocs", [E, o], F32, isOutput=False)
    out_d = nc.declare_dram_parameter("out", [tok, o], F32, isOutput=True)

    with tile.TileContext(nc) as tc:
        with (
            tc.tile_pool(name="const", bufs=1) as const,
            tc.tile_pool(name="gstg", bufs=1) as gstg,
            # ONE staging pool shared by x and w loads: 16KB/partition
            # tiles give 2MB transfers (16KB descriptors, ~+15% DMA rate
            # over 8KB), and the ring order enforces the startup priority
            # x half0 -> w stripe0 -> x half1 -> w stripe1 -> ...
            tc.tile_pool(name="stg", bufs=2) as stg,
            tc.tile_pool(name="wsgn", bufs=2) as wsgnp,
            tc.tile_pool(name="ossb", bufs=1) as ossbp,
            tc.tile_pool(name="outb", bufs=4) as outb,
            tc.tile_pool(name="pmm", bufs=TG, space="PSUM") as pmm,
            tc.tile_pool(name="pos", bufs=2, space="PSUM") as posp,
            tc.tile_pool(name="plg", bufs=2, space="PSUM") as plgp,
        ):
            # ---- persistent tiles ----
            # aT layout: [p, half, hc, t'] so each x half lands with one
            # contiguous cast-DMA
            aT = const.tile([P, TH * HC * 512], BF16, name="aT")
            aT4 = aT.rearrange("p (th hc t) -> p th hc t", th=TH, t=512)
            expT = const.tile([P, tok], BF16, name="expT")
            invden = const.tile([P, TB], F32, name="invden")
            invden2 = const.tile([P, TB], F32, name="invden2")
            ones_bf = const.tile([P, 1], BF16, name="ones_bf")
            gwT_bf = const.tile([P, HC * E], BF16, name="gwT_bf")
            ics_bf = const.tile([P, h], BF16, name="ics_bf")
            ocs_bf = const.tile([P, o], BF16, name="ocs_bf")
            bias_bc = const.tile([P, o], BF16, name="bias_bc")
            lsum = const.tile([E, 512], F32, name="lsum")

            nc.vector.memset(ones_bf, 1.0)
            nc.vector.memset(expT, 0.0)
            # padding memsets off the DVE critical path; FIFO order on the
            # SWDGE queue keeps them before the row-0:E loads
            nc.gpsimd.memset(ics_bf, 0.0)
            nc.gpsimd.memset(ocs_bf, 0.0)

            # gwT first on the scalar HWDGE ring (it gates the first logit
            # mm; the ACT ring starts ~8us sooner than the SP ring, which
            # carries Tile's startup semaphore traffic)
            gwT_f = gstg.tile([P, HC * E], F32, tag="gwstg", name="gwT_f")
            nc.scalar.dma_start(out=gwT_f, in_=gwT_d[:, :])
            nc.vector.tensor_copy(out=gwT_bf, in_=gwT_f)

            # x: raw f32 DMAs (cast-DMA runs ~4x below line rate, so cast
            # on DVE instead), XP parts per half so gating pipelines
            # behind the arrivals.  Half 0 rides the otherwise-idle sync
            # HWDGE ring; half 1 + small consts ride the SWDGE queue.
            HG = HC // XP
            def load_x_half(th, x_d, queue, cast):
                for g in range(XP):
                    xs = stg.tile([P, HG * 512], F32, tag="stg",
                                  name=f"xs_{th}_{g}")
                    queue(
                        out=xs,
                        in_=x_d[:, g * HG * 512 : (g + 1) * HG * 512],
                    )
                    cast(
                        out=aT4[:, th, g * HG : (g + 1) * HG, :], in_=xs
                    )

            load_x_half(0, x0_d, nc.scalar.dma_start, nc.vector.tensor_copy)
            nc.gpsimd.dma_start(out=ics_bf[0:E, :], in_=ics_d[:, :])
            nc.gpsimd.dma_start(out=ocs_bf[0:E, :], in_=ocs_d[:, :])

            # ---- w stripe loads + sign (scalar HWDGE queue + ACT) ----
            WG = HC // WP  # h-chunks per part
            def load_w_stripe(oc):
                ws = wsgnp.tile([P, HC * ON], BF16, tag="wsgn",
                                name=f"wsgn_{oc}")
                for g in range(WP):
                    wf = stg.tile([P, WG * ON], F32, tag="stg",
                                  name=f"wf_{oc}_{g}")
                    nc.scalar.dma_start(
                        out=wf,
                        in_=wr_d[:, (oc * HC + g * WG) * ON :
                                 (oc * HC + (g + 1) * WG) * ON],
                    )
                    nc.scalar.activation(
                        ws[:, g * WG * ON : (g + 1) * WG * ON], wf, AF.Sign
                    )
                return ws

            # ---- gating per 512-token half ----
            def gating_half(th):
                s0 = th * 512
                # split the 32-chunk logit accumulation across two PSUM
                # banks (a serial accumulate chain runs ~2x slower), then
                # merge on DVE before the exp
                pla = plgp.tile([E, 512], F32, tag="lg", name=f"pla_{th}")
                plb = plgp.tile([E, 512], F32, tag="lg", name=f"plb_{th}")
                for hc in range(HC):
                    pl = pla if hc % 2 == 0 else plb
                    nc.tensor.matmul(
                        pl,
                        gwT_bf[:, hc * E : (hc + 1) * E],
                        aT4[:, th, hc, :],
                        start=(hc < 2),
                        stop=(hc >= HC - 2),
                    )
                # DVE may read only one PSUM operand per instruction
                nc.vector.tensor_copy(out=lsum, in_=pla)
                nc.vector.tensor_tensor(lsum, lsum, plb, ALU.add)
                nc.scalar.activation(expT[0:E, s0 : s0 + 512], lsum, AF.Exp)
                for tb in range(th * 4, th * 4 + 4):
                    t0 = tb * P
                    # den written into col 0 of a pos-pool bank (PSUM is
                    # fully subscribed: 4 main + 2 os + 2 logit banks)
                    pdt = posp.tile([P, 512], F32, tag="os", name=f"pd_{tb}")
                    pd = pdt[:, 0:1]
                    nc.tensor.matmul(
                        pd, expT[:, t0 : t0 + P], ones_bf,
                        start=True, stop=True,
                    )
                    nc.vector.reciprocal(invden[:, tb : tb + 1], pd)
                    nc.vector.tensor_tensor(
                        invden2[:, tb : tb + 1],
                        invden[:, tb : tb + 1],
                        invden[:, tb : tb + 1],
                        ALU.mult,
                    )
                # fold is_raw into aT (in place)
                for hc in range(HC):
                    pis = posp.tile([P, 512], F32, tag="os",
                                    name=f"pis_{th}_{hc}")
                    nc.tensor.matmul(
                        pis,
                        ics_bf[:, hc * P : (hc + 1) * P],
                        expT[:, s0 : s0 + 512],
                        start=True,
                        stop=True,
                    )
                    sl = aT4[:, th, hc, :]
                    nc.vector.tensor_tensor(sl, sl, pis, ALU.mult)

            # exp(h0) must precede the stripe-0 SIGNs in the ACT FIFO, or
            # gating h0 stalls behind weight-data arrival
            gating_half(0)
            wsgn = load_w_stripe(0)
            # x half 1 after stripe 0 in the staging ring (it is needed
            # ~20us later than the first weights), bias last of all.
            # Casts stay on DVE: running them on GPSIMD serializes with
            # the doorbells on the Q7 FIFO and measured ~35us SLOWER.
            load_x_half(1, x1_d, nc.gpsimd.dma_start, nc.vector.tensor_copy)
            nc.gpsimd.dma_start(
                out=bias_bc, in_=b_d[None, :].to_broadcast((P, o))
            )

            # ---- main loop over output-column chunks ----
            for oc in range(OC):
                o0 = oc * ON
                os_sb = ossbp.tile([P, TB * ON], BF16, tag="ossb",
                                   name=f"ossb_{oc}")
                if 0 < oc < OC - 1:
                    wsgn_next = load_w_stripe(oc + 1)
                for tg in range(TB // TG):
                    if oc == 0 and tg == 1:
                        # second-half gating: on the PE queue right before
                        # the mains that consume it, and exp(h1) on the ACT
                        # FIFO before the stripe-1 SIGNs
                        gating_half(1)
                        wsgn_next = load_w_stripe(1)
                    tbs = list(range(tg * TG, (tg + 1) * TG))
                    # out_scale for this token group, 1/den^2 folded in
                    for tb in tbs:
                        pos = posp.tile([P, 512], F32, tag="os",
                                        name=f"pos_{oc}_{tb}")
                        nc.tensor.matmul(
                            pos,
                            expT[:, tb * P : (tb + 1) * P],
                            ocs_bf[:, o0 : o0 + 512],
                            start=True,
                            stop=True,
                        )
                        nc.vector.tensor_scalar_mul(
                            os_sb[:, tb * ON : (tb + 1) * ON],
                            pos,
                            invden2[:, tb : tb + 1],
                        )
                    last_tg = oc == OC - 1 and tg == (TB // TG) - 1

                    def epilogue(pm, tb, on_dve):
                        t0 = tb * P
                        tmp = outb.tile([P, ON], F32, tag="out")
                        nc.vector.tensor_tensor(
                            tmp, pm,
                            os_sb[:, tb * ON : (tb + 1) * ON],
                            ALU.mult,
                        )
                        if on_dve:
                            # tail: keep the final chain off the busy
                            # GPSIMD queue (DVE add + HWDGE store)
                            nc.vector.tensor_tensor(
                                tmp, tmp, bias_bc[:, o0 : o0 + 512], ALU.add
                            )
                            nc.sync.dma_start(
                                out=out_d[t0 : t0 + P, o0 : o0 + 512],
                                in_=tmp,
                            )
                        else:
                            nc.gpsimd.tensor_tensor(
                                tmp, tmp, bias_bc[:, o0 : o0 + 512], ALU.add
                            )
                            nc.gpsimd.dma_start(
                                out=out_d[t0 : t0 + P, o0 : o0 + 512],
                                in_=tmp,
                            )

                    def mains(group, stagger):
                        pms = [pmm.tile([P, ON], F32, tag="mm",
                                        name=f"pm_{oc}_{tg}_{tb}")
                               for tb in group]
                        for hc in range(HC):
                            for i, tb in enumerate(group):
                                t0 = (tb % 4) * P
                                nc.tensor.matmul(
                                    pms[i],
                                    aT4[:, tb // 4, hc, t0 : t0 + P],
                                    wsgn[:, hc * ON : (hc + 1) * ON],
                                    start=(hc == 0),
                                    stop=(hc == HC - 1),
                                )
                        for i, tb in enumerate(group):
                            epilogue(pms[i], tb, stagger)

                    if last_tg:
                        # split the final group into pairs so the first
                        # pair's epilogue overlaps the second pair's mains
                        mains(tbs[0:2], False)
                        mains(tbs[2:4], True)
                    else:
                        mains(tbs, False)
                if oc + 1 < OC:
                    wsgn = wsgn_next
    return nc


_NC_CACHE = {}


def _get_nc(key=None):
    if key is None:
        key = (TOK, FULL_H, FULL_O)
    if key not in _NC_CACHE:
        _NC_CACHE[key] = build_nc(*key)
    return _NC_CACHE[key]


def make_in_maps(x, weight, bias, gate_w, in_channel_scale, out_channel_scale):
    """Host-side sharding: token-slice x, and lay out x/w partition-major
    (pure permutations -- no arithmetic happens on the host)."""
    B, S, H = x.shape
    O = weight.shape[0]
    HC, ON = H // P, 512
    OC = O // ON
    xf = x.reshape(-1, H).astype(np.float32, copy=False)
    # [hc, p, t] -> [p, hc, t]
    xt = np.ascontiguousarray(
        xf.T.reshape(HC, P, -1).transpose(1, 0, 2))  # [128, 32, N]
    # w [o, h] -> wT[h, o] -> [p, oc, hc, o']
    wr = np.ascontiguousarray(
        weight.astype(np.float32, copy=False)
        .T.reshape(HC, P, OC, ON)
        .transpose(1, 2, 0, 3)
    ).reshape(P, -1)
    # gate_w [E, h] -> [p, hc, e]
    gwr = np.ascontiguousarray(
        gate_w.astype(np.float32, copy=False)
        .T.reshape(HC, P, E)
        .transpose(1, 0, 2)
    ).reshape(P, -1)
    bias = np.ascontiguousarray(bias.astype(np.float32, copy=False))
    ics = np.ascontiguousarray(
        in_channel_scale.astype(np.float32, copy=False))
    ocs = np.ascontiguousarray(
        out_channel_scale.astype(np.float32, copy=False))
    maps = []
    for c in range(N_CORES):
        sl = xt[:, :, c * TOK : (c + 1) * TOK]
        maps.append({
            "x0": np.ascontiguousarray(sl[:, :, 0:512]).reshape(P, -1),
            "x1": np.ascontiguousarray(sl[:, :, 512:1024]).reshape(P, -1),
            "wr": wr,
            "bias": bias,
            "gwr": gwr,
            "ics": ics,
            "ocs": ocs,
        })
    return maps


def kernel(x, weight, bias, gate_w, in_channel_scale, out_channel_scale):
    B, S, H = x.shape
    nc = _get_nc()
    in_maps = make_in_maps(
        x, weight, bias, gate_w, in_channel_scale, out_channel_scale
    )
    res = run_bass_kernel_spmd(nc, in_maps, list(range(N_CORES)))
    out = np.concatenate(
        [res.results[c]["out"] for c in range(N_CORES)], axis=0
    )
    return out.reshape(B, S, -1)



# revision 2
# speedup vs baseline: 1.5022x; 1.5022x over previous
"""BinaryMoSLinear Trainium2 kernel v2 (8-core SPMD, data-parallel tokens).

Math (per reference):
    xf      = x.reshape(N, H)
    routing = softmax(xf @ gate_w.T)            # [N, E], E = 8
    in_s    = routing @ in_channel_scale        # [N, H]
    out_s   = routing @ out_channel_scale       # [N, O]
    out     = (xf * in_s) @ sign(weight).T * out_s + bias

Device factorization (division-free; expT carries a 2^-S scale folded into
the ACT exp bias so the unnormalized activations fit fp8e4m3 range):
    expT[e, t]   = exp(logitsT[e, t] - S*ln2)   (bf16)
    den[t]       = sum_e expT[e, t]             (PE ones-mm, row layout)
    inv2[t]      = 1/den^2                      (broadcast to [128, t] via PE)
    a[h, t]      = x[h, t] * (ics @ expT)[h, t] (DVE; fp8 planes 0..2*NP8-1,
                                                 bf16 planes above)
    main[o, t]   = sum_h sign(w)[o, h] a[h, t]  (w-stationary; fp8 planes via
                                                 DoubleRow pairs, bf16 rest,
                                                 mixed accumulation per PSUM)
    out[o, t]    = main * (ocs @ expT)[o, t] * inv2[t] + bias[o]

The h-channels are PERMUTED host-side (pure layout, exact): channels with
the largest |in_channel_scale| magnitudes go to the bf16 planes so the fp8
quantization outliers land in bf16.  Output is produced [O, TOK] per core
and untransposed on the host.
"""

import numpy as np

import concourse.bass as bass
import concourse.mybir as mybir
from concourse import tile
from concourse.bass_utils import run_bass_kernel_spmd

F32 = mybir.dt.float32
BF16 = mybir.dt.bfloat16
FP8 = mybir.dt.float8e4
AF = mybir.ActivationFunctionType
ALU = mybir.AluOpType
DRM = mybir.MatmulPerfMode.DoubleRow

P = 128
E = 8
N_CORES = 8

FULL_B, FULL_S, FULL_H, FULL_O = 4, 2048, 4096, 4096
TOK = FULL_B * FULL_S // N_CORES  # 1024 tokens per core
TH = 2          # 512-token halves
HC = FULL_H // P  # 32 h-planes
NP8 = 8         # fp8 DoubleRow pair-groups (planes 0 .. 2*NP8-1)
NBF = HC - 2 * NP8  # bf16 planes
OCB = FULL_O // P   # 32 output chunks of 128
XP = 8          # x DMA parts per half
SEXP = 5.0      # expT scaled by 2^-SEXP (folded into ACT exp bias)
LN2 = 0.6931471805599453


# --------------------------------------------------------------------------
# This container's walrus build accepts at most ONE sync-wait command per
# instruction (DMA descriptors especially).  Tile's scheduler freely stacks
# several waits on one instruction, so rewrite the BIR JSON before compile:
# excess waits become single-wait NoOps immediately preceding the instruction
# on the same engine (program order => identical semantics).
_MAXW = 1


def _split_excess_waits(bir_json: bytes, maxw: int = _MAXW) -> bytes:
    import json as _json

    j = _json.loads(bir_json)
    ctr = 0
    for fn in j["functions"]:
        for blk in fn["blocks"]:
            new = []
            for inst in blk["instructions"]:
                si = inst.get("sync_info")
                if si:
                    waits = si.get("on_wait") or []
                    if len(waits) > maxw:
                        extra, keep = waits[:-maxw], waits[-maxw:]
                        for i in range(0, len(extra), maxw):
                            ctr += 1
                            nop = {
                                "name": f"I-wsplit-{ctr}",
                                "opcode": "NoOp",
                                "engine": inst["engine"],
                                "ins": [],
                                "outs": [],
                                "sync_info": {
                                    "on_wait": extra[i : i + maxw],
                                    "on_update": [],
                                },
                            }
                            if "debug" in inst:
                                nop["debug"] = inst["debug"]
                            new.append(nop)
                        si["on_wait"] = keep
                new.append(inst)
            blk["instructions"] = new
    return _json.dumps(j).encode()


def _install_wait_split():
    from concourse import bass2jax, bass_utils

    orig = bass_utils.compile_bir_kernel
    if getattr(orig, "_wait_split_wrapped", False):
        return

    def wrapped(bir_json, tmpdir, neff_name="file.neff"):
        return orig(_split_excess_waits(bir_json), tmpdir, neff_name)

    wrapped._wait_split_wrapped = True
    bass_utils.compile_bir_kernel = wrapped
    bass2jax.compile_bir_kernel = wrapped


_install_wait_split()
# --------------------------------------------------------------------------


def build_nc(tok=TOK, h=FULL_H, o=FULL_O):
    nc = bass.Bass("TRN2", target_bir_lowering=False, debug=False,
                   num_devices=N_CORES)

    # x halves, permuted to [p, hc, t'] (partition-major, contiguous).
    # Shipped as bf16: identical values to the device-side bf16 cast the
    # kernel would otherwise perform as its first op; halves x HBM traffic.
    x0_d = nc.declare_dram_parameter("x0", [P, HC * 512], BF16,
                                     isOutput=False)
    x1_d = nc.declare_dram_parameter("x1", [P, HC * 512], BF16,
                                     isOutput=False)
    # w permuted per ocb: [p, ocb, fp8(np8,j,o') 2048 | bf16(k,o') 2048].
    # Shipped bf16: only sign(w) is consumed and sign(bf16(w)) == sign(w)
    # (make_in_maps guards the measure-zero underflow-to-zero case).
    wr_d = nc.declare_dram_parameter("wr", [P, OCB * 4096], BF16,
                                     isOutput=False)
    bias_d = nc.declare_dram_parameter("biasc", [P, OCB], F32, isOutput=False)
    gwT_d = nc.declare_dram_parameter("gwr", [P, HC * E], F32, isOutput=False)
    ics_d = nc.declare_dram_parameter("ics", [E, h], F32, isOutput=False)
    ocs_d = nc.declare_dram_parameter("ocs", [E, o], F32, isOutput=False)
    out_d = nc.declare_dram_parameter("out", [o, tok], F32, isOutput=True)

    HG = HC // XP  # h-chunks per x part

    with tile.TileContext(nc) as tc:
        with (
            tc.tile_pool(name="const", bufs=1) as const,
            tc.tile_pool(name="gstg", bufs=1) as gstg,
            # ONE staging ring for x parts and w sub-chunks (all [P, 2048]
            # f32): ring order gives x absolute priority at startup, w
            # sub-chunks only begin once the x parts drain.
            tc.tile_pool(name="stg", bufs=6) as stg,
            tc.tile_pool(name="wcv", bufs=3) as wcv,
            tc.tile_pool(name="ossb", bufs=4) as ossbp,
            tc.tile_pool(name="outb", bufs=4) as outb,
            tc.tile_pool(name="pmm", bufs=4, space="PSUM") as pmm,
            tc.tile_pool(name="pos", bufs=2, space="PSUM") as posp,
            tc.tile_pool(name="plg", bufs=2, space="PSUM") as plgp,
        ):
            # ---- persistent tiles ----
            aT = const.tile([P, TH * HC * 512], BF16, name="aT")
            aT4 = aT.rearrange("p (th hc t) -> p th hc t", th=TH, t=512)
            a8 = const.tile([P, TH * NP8 * 2 * 512], FP8, name="a8")
            a84 = a8.rearrange("p (th i j t) -> p th i j t", th=TH, i=NP8,
                               j=2)
            expT = const.tile([P, tok], BF16, name="expT")
            ones_bf = const.tile([P, P], BF16, name="ones_bf")
            gwT_bf = const.tile([P, HC * E], BF16, name="gwT_bf")
            ics_bf = const.tile([P, h], BF16, name="ics_bf")
            ocs_bf = const.tile([P, o], BF16, name="ocs_bf")
            bias_sb = const.tile([P, OCB], F32, name="bias_sb")
            lsum = const.tile([E, 512], F32, name="lsum")
            invr = const.tile([1, TH * 512], F32, name="invr")
            inv2r = const.tile([1, TH * 512], F32, name="inv2r")
            inv2bc = const.tile([P, TH * 512], F32, name="inv2bc")

            ebias = const.tile([P, 1], F32, name="ebias")
            ones_f32 = const.tile([1, P], F32, name="ones_f32")
            nc.vector.memset(ebias, -SEXP * LN2)
            nc.vector.memset(ones_f32, 1.0)
            nc.vector.memset(ones_bf, 1.0)
            nc.vector.memset(expT, 0.0)
            # padding memsets off the DVE critical path; FIFO order on the
            # SWDGE queue keeps them before the row-0:E loads
            nc.gpsimd.memset(ics_bf, 0.0)
            nc.gpsimd.memset(ocs_bf, 0.0)

            # gwT first on the scalar HWDGE ring (it gates the first logit
            # mm)
            gwT_f = gstg.tile([P, HC * E], F32, tag="gwstg", name="gwT_f")
            nc.scalar.dma_start(out=gwT_f, in_=gwT_d[:, :])
            nc.vector.tensor_copy(out=gwT_bf, in_=gwT_f)

            # x parts: bf16 DMAs straight into aT (no staging, no casts);
            # x0 on the scalar ring, x1 on the sync ring, in parallel.
            # Ring issue order puts all x before the w sub-chunks.
            for g in range(XP):
                nc.scalar.dma_start(
                    out=aT4[:, 0, g * HG : (g + 1) * HG, :],
                    in_=x0_d[:, g * HG * 512 : (g + 1) * HG * 512],
                )
                nc.sync.dma_start(
                    out=aT4[:, 1, g * HG : (g + 1) * HG, :],
                    in_=x1_d[:, g * HG * 512 : (g + 1) * HG * 512],
                )

            nc.gpsimd.dma_start(out=ics_bf[0:E, :], in_=ics_d[:, :])
            nc.gpsimd.dma_start(out=ocs_bf[0:E, :], in_=ocs_d[:, :])
            nc.gpsimd.dma_start(out=bias_sb, in_=bias_d[:, :])

            # PE warmup: ~60 throwaway mms on zeroed tiles keep the PE
            # continuously busy while x streams in, so the HAM clock-gate
            # reaches 8/8 before gating -- otherwise every gating matmul
            # runs at the cold 1.2GHz rate (measured ~600ns vs 213ns).
            pwarm = pmm.tile([P, 512], F32, tag="mm", name="pwarm")
            for _ in range(60):
                nc.tensor.matmul(
                    pwarm, ones_bf, expT[:, 0:512],
                    start=True, stop=True,
                )

            # ---- gating, th0/th1 interleaved ----
            # logits: one PSUM accumulator per half; alternating banks per
            # mm avoids the serial-accumulate stall, and both x DMA rings
            # pace the PE in parallel.
            plas = [plgp.tile([E, 512], F32, tag="lg", name=f"pla_{th}")
                    for th in range(TH)]
            for hc in range(HC):
                for th in range(TH):
                    nc.tensor.matmul(
                        plas[th],
                        gwT_bf[:, hc * E : (hc + 1) * E],
                        aT4[:, th, hc, :],
                        start=(hc == 0),
                        stop=(hc == HC - 1),
                    )

            def post_logits(th):
                s0 = th * 512
                # expT = exp(logits - SEXP*ln2), read directly from PSUM:
                # the 2^-SEXP scale keeps the unnormalized activations
                # inside fp8e4m3 range; exactly compensated by
                # inv2 = (2^SEXP/den)^2 net scaling.
                nc.scalar.activation(
                    expT[0:E, s0 : s0 + 512], plas[th], AF.Exp,
                    bias=ebias[0:E, :],
                )
                # den as a row: [1, 512] = ones[k=128(e-pad), 1].T @ expT
                pden = posp.tile([P, 512], F32, tag="os", name=f"pden_{th}")
                nc.tensor.matmul(
                    pden[0:1, :],
                    ones_bf[:, 0:1],
                    expT[:, s0 : s0 + 512],
                    start=True, stop=True,
                )
                nc.vector.reciprocal(invr[0:1, s0 : s0 + 512], pden[0:1, :])
                nc.vector.tensor_tensor(
                    inv2r[0:1, s0 : s0 + 512],
                    invr[0:1, s0 : s0 + 512],
                    invr[0:1, s0 : s0 + 512],
                    ALU.mult,
                )
                # broadcast inv2 row to all 128 partitions via K=1 f32 mm
                pbc = posp.tile([P, 512], F32, tag="os", name=f"pbc_{th}")
                nc.tensor.matmul(
                    pbc,
                    ones_f32[0:1, :],
                    inv2r[0:1, s0 : s0 + 512],
                    start=True, stop=True,
                )
                nc.vector.tensor_copy(
                    out=inv2bc[:, s0 : s0 + 512], in_=pbc
                )

            post_logits(0)
            post_logits(1)

            # fold in_scale into activations: fp8 planes -> a8, bf16
            # planes -> aT in place (DVE only: GPSIMD cannot read PSUM)
            for th in range(TH):
                s0 = th * 512
                for hc in range(HC):
                    pis = posp.tile([P, 512], F32, tag="os",
                                    name=f"pis_{th}_{hc}")
                    nc.tensor.matmul(
                        pis,
                        ics_bf[:, hc * P : (hc + 1) * P],
                        expT[:, s0 : s0 + 512],
                        start=True, stop=True,
                    )
                    sl = aT4[:, th, hc, :]
                    if hc < 2 * NP8:
                        nc.vector.tensor_tensor(
                            a84[:, th, hc // 2, hc % 2, :], sl, pis,
                            ALU.mult,
                        )
                    else:
                        nc.vector.tensor_tensor(sl, sl, pis, ALU.mult)

            # ---- w load + sign conversion (per output chunk of 128) ----
            def load_w(ocb):
                ws = stg.tile([P, 4096], BF16, tag="stg", name=f"wf_{ocb}")
                nc.scalar.dma_start(
                    out=ws, in_=wr_d[:, ocb * 4096 : (ocb + 1) * 4096]
                )
                return ws

            def sign_w(ws, ocb):
                w8t = wcv.tile([P, NP8 * 2 * P], FP8, tag="w8",
                               name=f"w8_{ocb}")
                nc.scalar.activation(w8t, ws[:, 0:2048], AF.Sign)
                wbt = wcv.tile([P, NBF * P], BF16, tag="wb",
                               name=f"wb_{ocb}")
                nc.scalar.activation(wbt, ws[:, 2048:4096], AF.Sign)
                return (
                    w8t.rearrange("p (i j m) -> p i j m", i=NP8, j=2),
                    wbt,
                )

            parts = load_w(0)
            parts_next = load_w(1)
            w8v, wbt = sign_w(parts, 0)

            # ---- main loop over output chunks ----
            for ocb in range(OCB):
                parts_fut = load_w(ocb + 2) if ocb + 2 < OCB else None
                if ocb + 1 < OCB:
                    w8v_n, wbt_n = sign_w(parts_next, ocb + 1)
                # out_scale for both halves, inv2 folded in
                os_sbs = []
                for th in range(TH):
                    s0 = th * 512
                    pos2 = posp.tile([P, 512], F32, tag="os",
                                     name=f"pos_{ocb}_{th}")
                    nc.tensor.matmul(
                        pos2,
                        ocs_bf[:, ocb * P : (ocb + 1) * P],
                        expT[:, s0 : s0 + 512],
                        start=True, stop=True,
                    )
                    os_sb = ossbp.tile([P, 512], F32, tag="os",
                                       name=f"ossb_{ocb}_{th}")
                    nc.vector.tensor_tensor(
                        os_sb, pos2, inv2bc[:, s0 : s0 + 512], ALU.mult
                    )
                    os_sbs.append(os_sb)
                # mains: th0/th1 interleaved per stationary so each
                # LDWEIGHTS hides under two 512-wide streams
                def main_mms(pm, th):
                    for i in range(NP8):
                        nc.tensor.matmul(
                            pm,
                            w8v[:, i, :, :],
                            a84[:, th, i, :, :],
                            start=(i == 0), stop=False,
                            perf_mode=DRM,
                        )
                    for k in range(NBF):
                        nc.tensor.matmul(
                            pm,
                            wbt[:, k * P : (k + 1) * P],
                            aT4[:, th, 2 * NP8 + k, :],
                            start=False, stop=(k == NBF - 1),
                        )

                # epilogue (both ops DVE: gpsimd tensor_scalar measured
                # 7.5us per 65K-elem op -- 6x slower than DVE)
                def epilogue(pm, th):
                    s0 = th * 512
                    tmp = outb.tile([P, 512], F32, tag="out")
                    nc.vector.tensor_tensor(tmp, pm, os_sbs[th], ALU.mult)
                    nc.vector.tensor_scalar_add(
                        tmp, tmp, bias_sb[:, ocb : ocb + 1]
                    )
                    if ocb >= OCB - 2:
                        # w rings are drained by now; keep the tail off the
                        # backlogged gpsimd queue
                        q = nc.scalar.dma_start if th == 0 else \
                            nc.sync.dma_start
                    else:
                        q = nc.gpsimd.dma_start if th == 0 else \
                            nc.sync.dma_start
                    q(
                        out=out_d[ocb * P : (ocb + 1) * P, s0 : s0 + 512],
                        in_=tmp,
                    )

                pms = [pmm.tile([P, 512], F32, tag="mm",
                                name=f"pm_{ocb}_{th}") for th in range(TH)]
                if ocb < OCB - 1:
                    for i in range(NP8):
                        for th in range(TH):
                            nc.tensor.matmul(
                                pms[th],
                                w8v[:, i, :, :],
                                a84[:, th, i, :, :],
                                start=(i == 0), stop=False,
                                perf_mode=DRM,
                            )
                    for k in range(NBF):
                        for th in range(TH):
                            nc.tensor.matmul(
                                pms[th],
                                wbt[:, k * P : (k + 1) * P],
                                aT4[:, th, 2 * NP8 + k, :],
                                start=False, stop=(k == NBF - 1),
                            )
                    for th in range(TH):
                        epilogue(pms[th], th)
                else:
                    # tail: de-interleave so th0's epilogue + store hide
                    # under th1's mains
                    main_mms(pms[0], 0)
                    epilogue(pms[0], 0)
                    main_mms(pms[1], 1)
                    epilogue(pms[1], 1)
                if ocb + 1 < OCB:
                    w8v, wbt = w8v_n, wbt_n
                    parts_next = parts_fut
    return nc


_NC_CACHE = {}


def _get_nc(key=None):
    if key is None:
        key = (TOK, FULL_H, FULL_O)
    if key not in _NC_CACHE:
        _NC_CACHE[key] = build_nc(*key)
    return _NC_CACHE[key]


def _channel_perm(in_channel_scale):
    """Pure layout choice: order h-channels so the largest
    |in_channel_scale| magnitudes land in the bf16 planes (the tail)."""
    chmag = np.abs(in_channel_scale).max(axis=0)
    return np.argsort(chmag, kind="stable")


def make_in_maps(x, weight, bias, gate_w, in_channel_scale, out_channel_scale):
    """Host-side sharding: token-slice x, permute channels, lay out
    partition-major (pure permutations -- no value arithmetic)."""
    B, S, H = x.shape
    O = weight.shape[0]
    perm = _channel_perm(in_channel_scale)

    bf16 = mybir.dt.np(BF16)
    xf = x.reshape(-1, H).astype(np.float32, copy=False)[:, perm]
    wp = weight.astype(np.float32, copy=False)[:, perm]
    gwp = gate_w.astype(np.float32, copy=False)[:, perm]
    icsp = np.ascontiguousarray(
        in_channel_scale.astype(np.float32, copy=False)[:, perm])
    ocs = np.ascontiguousarray(
        out_channel_scale.astype(np.float32, copy=False))

    # x -> [p, hc, t], bf16 (same rounding as the device-side cast)
    xt = np.ascontiguousarray(
        xf.T.reshape(HC, P, -1).transpose(1, 0, 2)).astype(bf16)
    # w fp8 part: [O, 2*NP8*128] -> [i, j, p, ocb, o'] -> [p, ocb, i, j, o']
    w8 = (wp.T[: 2 * NP8 * P, :]
          .reshape(NP8, 2, P, OCB, P)
          .transpose(2, 3, 0, 1, 4)
          .reshape(P, OCB, 2048))
    # w bf16 part: [k, p, ocb, o'] -> [p, ocb, k, o']
    wb = (wp.T[2 * NP8 * P :, :]
          .reshape(NBF, P, OCB, P)
          .transpose(1, 2, 0, 3)
          .reshape(P, OCB, 2048))
    wr = np.ascontiguousarray(
        np.concatenate([w8, wb], axis=2)).reshape(P, -1)
    wrb = wr.astype(bf16)
    # guard the (measure-zero) bf16 underflow-to-zero: sign must survive
    zm = (wrb.astype(np.float32) == 0.0) & (wr != 0.0)
    if zm.any():
        wrb[zm] = (np.sign(wr[zm]) * 1e-38).astype(bf16)
    wr = wrb
    # gate_w -> [p, hc, e]
    gwr = np.ascontiguousarray(
        gwp.T.reshape(HC, P, E).transpose(1, 0, 2)).reshape(P, -1)
    biasc = np.ascontiguousarray(
        bias.astype(np.float32, copy=False).reshape(OCB, P).T)

    maps = []
    for c in range(N_CORES):
        sl = xt[:, :, c * TOK : (c + 1) * TOK]
        maps.append({
            "x0": np.ascontiguousarray(sl[:, :, 0:512]).reshape(P, -1),
            "x1": np.ascontiguousarray(sl[:, :, 512:1024]).reshape(P, -1),
            "wr": wr,
            "biasc": biasc,
            "gwr": gwr,
            "ics": icsp,
            "ocs": ocs,
        })
    return maps


def kernel(x, weight, bias, gate_w, in_channel_scale, out_channel_scale):
    B, S, H = x.shape
    nc = _get_nc()
    in_maps = make_in_maps(
        x, weight, bias, gate_w, in_channel_scale, out_channel_scale
    )
    res = run_bass_kernel_spmd(nc, in_maps, list(range(N_CORES)))
    out = np.concatenate(
        [res.results[c]["out"].T for c in range(N_CORES)], axis=0
    )
    return out.reshape(B, S, -1)
